# revision 6
# baseline (speedup 1.0000x reference)
"""HGAT message-passing kernel for Trainium2 (8 NeuronCores, SPMD).

Reference computation (B=4, N=4096, C_IN=128, C_OUT=64):
    h   = node_rep @ proj_W.T + proj_b                    # [B,N,64]
    f1  = rowsum(h * k_W[node_type]) + k_b[node_type]     # [B,N]
    f2  = rowsum(h * v_W[node_type]) + v_b[node_type]     # [B,N]
    L   = adj[i,j] * (f1[i] + f2[j])
    u   = sigmoid(L) - 0.5
    P   = softmax(u, axis=i)      # normalized over rows i, per column j
    out = P @ h                   # contract over j

Key algebra used on device:
  * softmax-over-i / contract-over-j means out = E @ (h / colsum) with
    E[i,j] = exp(sigmoid(L)) and colsum[j] = sum_i E[i,j]; the -0.5 and the
    softmax max-subtraction cancel in the ratio.
  * exp(sigmoid(x)) ~= D + A*sigmoid(B*x + C) with max rel err 4.1e-4, so
    ONE ACT pass (Sigmoid, accum_out -> colsum) gives s'; the B scale folds
    into host-prescaled k/v params, C is the ACT bias, A and the rank-1
    D-term fold out on the host combine.
  * L' = (f1[i] + f2[j]) * adjT[j,i] is ONE scalar_tensor_tensor pass on
    the DVE (per-partition scalar f2, 16-bit 2x mode) — no separate f12.
  * f2 rowsum fuses into one STT with accum_out.
  * final matmul is transposed (out_T[o,i] = g.T @ s'), bf16 both sides;
    the D-term needs only sum_j g[j,:], computed on-device by a ones
    matmul (sgp output, [1, NJT*COUT]), so g never travels to the host.
  * adjacency travels as bf16 (tolerance 2e-2; quantization adds ~1e-4).

Sharding: core c handles batch b=c//2 and j-half h=c%2 (rows of adj.T).
The i axis is rolled per-core so the core's own j columns sit first in
xt — hn then reads fixed xt slices; the host un-rolls outp at the end.
Host prep is layout/cast/gather + param prescale only.
"""

import os
import sys

import numpy as np

sys.path.insert(0, "/opt/trn_rl_repo")

import ml_dtypes  # noqa: E402

import concourse.tile as tile  # noqa: E402
from concourse import bacc  # noqa: E402
from concourse import mybir  # noqa: E402
from concourse.bass_utils import run_bass_kernel_spmd  # noqa: E402

B = 4
N = 4096
CIN = 128
COUT = 64
P = 128                      # SBUF partitions
NJ = N // 2                  # j rows per core (adjacency half)
NJT = NJ // P                # 16 j-tiles per core
NIC = N // 512               # 8 i-chunks of 512

F32 = mybir.dt.float32
BF16 = mybir.dt.bfloat16
AF = mybir.ActivationFunctionType
ALU = mybir.AluOpType

# exp(sigmoid(x)) ~= FIT_D + FIT_A * sigmoid(FIT_B * x + FIT_C)
FIT_A = 1.71677394
FIT_B = 1.01816816
FIT_C = -0.49959447
FIT_D = 1.00040553

LAST_EXEC_NS = None
LAST_RESULTS = None


def build_nc():
    """Single-core SPMD Bass program (same program on all cores)."""
    nc = bacc.Bacc()
    adjt_d = nc.dram_tensor("adjt", [NJ, N], BF16, kind="ExternalInput")
    xt_d = nc.dram_tensor("xt", [CIN, N], BF16, kind="ExternalInput")
    wpt_d = nc.dram_tensor("wpt", [CIN, COUT], BF16, kind="ExternalInput")
    bpcol_d = nc.dram_tensor("bpcol", [COUT, 1], F32, kind="ExternalInput")
    bpb_d = nc.dram_tensor("bpb", [P, COUT], F32, kind="ExternalInput")
    kwt_d = nc.dram_tensor("kwt", [COUT, N], BF16, kind="ExternalInput")
    kbrow_d = nc.dram_tensor("kbrow", [1, N], BF16, kind="ExternalInput")
    vwn_d = nc.dram_tensor("vwn", [P, NJT * COUT], BF16, kind="ExternalInput")
    vbcol_d = nc.dram_tensor("vbcol", [P, NJT], F32, kind="ExternalInput")
    ones65_d = nc.dram_tensor("ones65", [COUT + 1, P], BF16, kind="ExternalInput")
    ones1_d = nc.dram_tensor("ones1", [P, 1], BF16, kind="ExternalInput")
    outp_d = nc.dram_tensor("outp", [COUT, N], F32, kind="ExternalOutput")
    sgp_d = nc.dram_tensor("sgp", [1, NJT * COUT], F32, kind="ExternalOutput")

    with tile.TileContext(nc) as tc:
        with (
            tc.tile_pool(name="singles", bufs=1) as singles,
            tc.tile_pool(name="f1s", bufs=8) as f1s,
            tc.tile_pool(name="stream", bufs=3) as stream,
            tc.tile_pool(name="adjp", bufs=3) as adjp,
            tc.tile_pool(name="lp", bufs=3) as lp,
            tc.tile_pool(name="etp", bufs=3) as etp,
            tc.tile_pool(name="smalls", bufs=4) as smalls,
        ):
            # ---------------- parameter loads ----------------
            # Small params on the scalar-engine HWDGE ring; big streams on
            # the sync ring so issue isn't serialized on one queue.
            wpt_s = singles.tile([CIN, COUT], BF16)
            nc.scalar.dma_start(wpt_s, wpt_d[:, :])
            bpcol_s = singles.tile([COUT, 1], F32)
            nc.scalar.dma_start(bpcol_s, bpcol_d[:, :])
            ones65 = singles.tile([COUT + 1, P], BF16)
            nc.scalar.dma_start(ones65, ones65_d[:, :])
            bpb_s = singles.tile([P, COUT], F32)
            nc.scalar.dma_start(bpb_s, bpb_d[:, :])
            vbcol_s = singles.tile([P, NJT], F32)
            nc.scalar.dma_start(vbcol_s, vbcol_d[:, :])
            vwn_s = singles.tile([P, NJT * COUT], BF16)
            nc.scalar.dma_start(vwn_s, vwn_d[:, :])
            ones1 = singles.tile([P, 1], BF16)
            nc.scalar.dma_start(ones1, ones1_d[:, :])
            cbias = singles.tile([P, 1], F32)
            nc.vector.memset(cbias, FIT_C)

            f1b = singles.tile([P, N], BF16)
            hn = singles.tile([P, NJT * COUT], F32)
            f2c = singles.tile([P, NJT], F32)
            f2cb = singles.tile([P, NJT], F32)
            g_all = singles.tile([P, NJT * COUT], BF16)

            # f1-path streams: all chunk DMAs issued upfront on sync ring
            xtcs = []
            kwcs = []
            for ic in range(NIC):
                sl = slice(ic * 512, (ic + 1) * 512)
                xtc = f1s.tile([CIN, 512], BF16, tag="xtc")
                nc.sync.dma_start(xtc, xt_d[:, sl])
                kwc = f1s.tile([COUT, 512], BF16, tag="kwc")
                nc.sync.dma_start(kwc, kwt_d[:, sl])
                xtcs.append(xtc)
                kwcs.append(kwc)
            # first adjacency tiles right behind them on the same ring
            adj_tiles = {}
            for jt in range(2):
                adjt_t = adjp.tile([P, N], BF16, tag="adj")
                nc.sync.dma_start(adjt_t, adjt_d[jt * P:(jt + 1) * P, :])
                adj_tiles[jt] = adjt_t

            sigmas = []  # (jt, cs, sp) pending post-sigma work

            # ---------------- pre + L/sigmoid phase (pre PSUM scoped) ------
            with (
                tc.tile_pool(name="psPreA", bufs=2, space="PSUM") as psA,
                tc.tile_pool(name="psPreB", bufs=2, space="PSUM") as psB,
            ):
                # f1 row: per chunk hT matmul, (hT+bp)*kw' STT, and a K=65
                # matmul against all-ones [65,P] that reduces over o, adds
                # kb (carried in row 64), and broadcasts to all partitions.
                for ic in range(NIC):
                    sl = slice(ic * 512, (ic + 1) * 512)
                    psh = psA.tile([COUT, 512], F32, tag="psh")
                    nc.tensor.matmul(
                        psh, lhsT=wpt_s, rhs=xtcs[ic], start=True, stop=True
                    )
                    prod = stream.tile([COUT + 1, 512], BF16, tag="prod")
                    nc.scalar.dma_start(prod[COUT:COUT + 1, :], kbrow_d[:, sl])
                    nc.vector.scalar_tensor_tensor(
                        prod[0:COUT, :], psh, bpcol_s, kwcs[ic],
                        op0=ALU.add, op1=ALU.mult,
                    )
                    psb = psA.tile([P, 512], F32, tag="psb")
                    nc.tensor.matmul(psb, lhsT=ones65, rhs=prod, start=True, stop=True)
                    # ACT is idle in the pre-phase; keep the DVE free for
                    # the prod STTs so f1b completes sooner.
                    nc.scalar.copy(f1b[:, sl], psb)

                def hn_chain(t):
                    # h for j-tile t from xt slices (host rolled xt so this
                    # core's j-half is columns [0, NJ)); fused f2 via STT
                    # accum.
                    osl = slice(t * COUT, (t + 1) * COUT)
                    c, off = divmod(t * P, 512)
                    psn = psB.tile([P, COUT], F32, tag="psn")
                    nc.tensor.matmul(
                        psn, lhsT=xtcs[c][:, off:off + P], rhs=wpt_s,
                        start=True, stop=True,
                    )
                    nc.vector.tensor_add(hn[:, osl], psn, bpb_s)
                    dump = smalls.tile([P, COUT], BF16, tag="dump")
                    nc.vector.scalar_tensor_tensor(
                        dump, hn[:, osl], 0.0, vwn_s[:, osl],
                        op0=ALU.add, op1=ALU.mult,
                        accum_out=f2c[:, t:t + 1],
                    )
                    nc.vector.tensor_scalar_add(
                        f2cb[:, t:t + 1], f2c[:, t:t + 1], vbcol_s[:, t:t + 1]
                    )

                hn_chain(0)
                # L + sigmoid stream; hn for tile t+1 interleaves so the
                # DVE is one hn-chain ahead of its own L pass.
                for jt in range(NJT):
                    if jt in adj_tiles:
                        adjt_t = adj_tiles.pop(jt)
                    else:
                        adjt_t = adjp.tile([P, N], BF16, tag="adj")
                        nc.sync.dma_start(adjt_t, adjt_d[jt * P:(jt + 1) * P, :])
                    if jt + 1 < NJT:
                        hn_chain(jt + 1)
                    # L' = (f1' + f2'[j]) * adjT in ONE DVE pass (16-bit 2x)
                    lt = lp.tile([P, N], BF16, tag="lt")
                    nc.vector.scalar_tensor_tensor(
                        lt, f1b, f2cb[:, jt:jt + 1], adjt_t,
                        op0=ALU.add, op1=ALU.mult,
                    )
                    # s' = sigmoid(L' + C), accum -> per-j partial colsum
                    sp = etp.tile([P, N], BF16, tag="sp")
                    cs = smalls.tile([P, 1], F32, tag="cs")
                    nc.scalar.activation(sp, lt, AF.Sigmoid, bias=cbias, accum_out=cs)
                    sigmas.append((jt, cs, sp))

            # ---------------- accumulation over j-tiles ----------------
            out_sb = singles.tile([COUT, N], F32)
            with tc.tile_pool(name="psMain", bufs=1, space="PSUM") as psM:
                ps_out = psM.tile([COUT, N], F32)
                for jt, cs, sp in sigmas:
                    # colsum = D*N + A*acc ; g = h/colsum ; out_T += g.T @ s'
                    t1 = smalls.tile([P, 1], F32, tag="t1")
                    nc.vector.tensor_scalar(
                        t1, cs, FIT_A, float(FIT_D * N), op0=ALU.mult, op1=ALU.add
                    )
                    rc = smalls.tile([P, 1], F32, tag="rc")
                    nc.vector.reciprocal(rc, t1)
                    gsl = slice(jt * COUT, (jt + 1) * COUT)
                    nc.vector.tensor_scalar_mul(g_all[:, gsl], hn[:, gsl], rc)
                    for c in range(NIC):
                        csl = slice(c * 512, (c + 1) * 512)
                        nc.tensor.matmul(
                            ps_out[:, csl],
                            lhsT=g_all[:, gsl],
                            rhs=sp[:, csl],
                            start=(jt == 0),
                            stop=(jt == NJT - 1),
                        )
                        if jt == NJT - 1:
                            # PSUM is not DMA-able: stage through SBUF,
                            # alternating engines, DMA per chunk.
                            if c % 2 == 0:
                                nc.vector.tensor_copy(out_sb[:, csl], ps_out[:, csl])
                            else:
                                nc.scalar.copy(out_sb[:, csl], ps_out[:, csl])
                            nc.sync.dma_start(outp_d[:, csl], out_sb[:, csl])

            # sum_j g[j,:] via ones matmul -> sgp (D-term, host combine)
            with tc.tile_pool(name="psSg", bufs=2, space="PSUM") as psS:
                sg_sb = singles.tile([1, NJT * COUT], F32)
                for hgi in range(2):
                    hsl = slice(hgi * 512, (hgi + 1) * 512)
                    ps_sg = psS.tile([1, 512], F32, tag="sg")
                    nc.tensor.matmul(
                        ps_sg, lhsT=ones1, rhs=g_all[:, hsl], start=True, stop=True
                    )
                    nc.vector.tensor_copy(sg_sb[:, hsl], ps_sg)
                nc.scalar.dma_start(sgp_d[:, :], sg_sb)

    nc.finalize()
    return nc


def _prep_in_maps(node_rep, adj_matrix, node_type, proj_W, proj_b, k_W, k_b, v_W, v_b):
    """Host-side shard prep (layout/cast/gather only, no model math)."""
    f32 = np.float32
    bf = ml_dtypes.bfloat16
    node_rep = np.asarray(node_rep, dtype=f32)
    adj = np.asarray(adj_matrix, dtype=f32)
    nt = np.asarray(node_type).astype(np.int64) % 5
    proj_W = np.asarray(proj_W, dtype=f32)
    proj_b = np.asarray(proj_b, dtype=f32)
    k_W = np.asarray(k_W, dtype=f32) * f32(FIT_B)
    k_b = np.asarray(k_b, dtype=f32) * f32(FIT_B)
    v_W = np.asarray(v_W, dtype=f32) * f32(FIT_B)
    v_b = np.asarray(v_b, dtype=f32) * f32(FIT_B)

    adjT = np.ascontiguousarray(adj.T.astype(bf))            # [j, i] bf16
    wpt = np.ascontiguousarray(proj_W.T.astype(bf))          # [CIN, COUT]
    bpcol = np.ascontiguousarray(proj_b[:, None])            # [COUT, 1]
    bpb = np.ascontiguousarray(np.broadcast_to(proj_b[None, :], (P, COUT)))
    kwt = np.ascontiguousarray(k_W[nt].T.astype(bf))         # [COUT, N]
    kbrow = np.ascontiguousarray(k_b[nt][None, :].astype(bf))  # [1, N]
    VW = v_W[nt]                                             # [N, COUT]
    vb = v_b[nt]                                             # [N]

    in_maps = []
    for core in range(8):
        b, half = divmod(core, 2)
        jsl = slice(half * NJ, (half + 1) * NJ)
        xT = np.ascontiguousarray(node_rep[b].T.astype(bf))  # [CIN, N]
        # roll the i axis so this core's j-half occupies columns [0, NJ):
        # hn then indexes xt at fixed offsets; outp is un-rolled on host.
        xTr = np.ascontiguousarray(np.roll(xT, -half * NJ, axis=1))
        kwtr = np.ascontiguousarray(np.roll(kwt, -half * NJ, axis=1))
        kbrowr = np.ascontiguousarray(np.roll(kbrow, -half * NJ, axis=1))
        adjr = np.ascontiguousarray(np.roll(adjT[jsl, :], -half * NJ, axis=1))
        vw_h = VW[jsl]                                       # [NJ, COUT]
        vwn = np.ascontiguousarray(
            vw_h.reshape(NJT, P, COUT).transpose(1, 0, 2).reshape(P, NJT * COUT)
            .astype(bf)
        )
        vbcol = np.ascontiguousarray(vb[jsl].reshape(NJT, P).T)  # [P, NJT]
        in_maps.append({
            "adjt": adjr,
            "xt": xTr,
            "wpt": wpt,
            "bpcol": bpcol,
            "bpb": bpb,
            "kwt": kwtr,
            "kbrow": kbrowr,
            "vwn": vwn,
            "vbcol": vbcol,
            "ones65": np.ones((COUT + 1, P), dtype=bf),
            "ones1": np.ones((P, 1), dtype=bf),
        })
    return in_maps


def kernel(node_rep, adj_matrix, node_type, proj_W, proj_b, k_W, k_b, v_W, v_b):
    global LAST_EXEC_NS, LAST_RESULTS
    in_maps = _prep_in_maps(
        node_rep, adj_matrix, node_type, proj_W, proj_b, k_W, k_b, v_W, v_b
    )
    nc = build_nc()
    trace = os.environ.get("KERNEL_TRACE", "0") == "1"
    res = run_bass_kernel_spmd(nc, in_maps, core_ids=list(range(8)), trace=trace)
    LAST_EXEC_NS = res.exec_time_ns
    LAST_RESULTS = res

    out = np.empty((B, N, COUT), dtype=np.float32)
    for b in range(B):
        m = None
        sg = None
        for half in range(2):
            r = res.results[2 * b + half]
            mp = np.asarray(r["outp"], dtype=np.float32)          # [COUT, N]
            mp = np.roll(mp, half * NJ, axis=1)  # un-roll the i axis
            sp = np.asarray(r["sgp"], dtype=np.float32).reshape(NJT, COUT).sum(axis=0)
            m = mp if m is None else m + mp
            sg = sp if sg is None else sg + sp
        out[b] = FIT_A * m.T + FIT_D * sg[None, :]
    return out


# revision 11
# speedup vs baseline: 1.2017x; 1.2017x over previous
"""HGAT message-passing kernel for Trainium2 (8 NeuronCores, SPMD).

Reference computation (B=4, N=4096, C_IN=128, C_OUT=64):
    h   = node_rep @ proj_W.T + proj_b                    # [B,N,64]
    f1  = rowsum(h * k_W[node_type]) + k_b[node_type]     # [B,N]
    f2  = rowsum(h * v_W[node_type]) + v_b[node_type]     # [B,N]
    L   = adj[i,j] * (f1[i] + f2[j])
    u   = sigmoid(L) - 0.5
    P   = softmax(u, axis=i)      # normalized over rows i, per column j
    out = P @ h                   # contract over j

Key algebra used on device:
  * softmax-over-i / contract-over-j means out = E @ (h / colsum) with
    E[i,j] = exp(sigmoid(L)) and colsum[j] = sum_i E[i,j]; the -0.5 and the
    softmax max-subtraction cancel in the ratio.
  * exp(sigmoid(x)) ~= D + A*sigmoid(B*x + C) with max rel err 4.1e-4, so
    ONE ACT pass (Sigmoid, accum_out -> colsum) gives s'; the B scale folds
    into host-prescaled k/v params, C is the ACT bias, A and the rank-1
    D-term fold out on the host combine.
  * f12 = f1 + f2[j] is a DVE tensor_scalar (4x bf16); L' = f12*adjT a DVE
    tensor_tensor (2x bf16). (A fused scalar_tensor_tensor runs at 1x on
    HW — slower than the pair.)
  * f2 rowsum fuses into one small STT with accum_out.
  * final matmul is transposed (out_T[o,i] = g.T @ s'), bf16 both sides;
    the D-term needs only sum_j g[j,:], computed on-device by a ones
    matmul (sgp output), so g never travels to the host.
  * adjacency travels as bf16 (tolerance 2e-2; quantization adds ~1e-4).

Sharding: core c handles batch b=c//2 and j-half h=c%2 (rows of adj.T).
The i axis is rolled per-core so the core's own j columns sit first in
xt — hn then reads fixed xt slices; the host un-rolls outp at the end.

DMA issue costs ~600ns of queue time each, so transfers are batched:
xt in 2, kwt/kbrow/params in 1 each (scalar ring), adjacency as 8 x 2MB
double-tile loads (sync ring). Output staging copies alternate DVE/ACT.
"""

import os
import sys

import numpy as np

sys.path.insert(0, "/opt/trn_rl_repo")

import ml_dtypes  # noqa: E402

import concourse.tile as tile  # noqa: E402
from concourse import bacc  # noqa: E402
from concourse import mybir  # noqa: E402
from concourse.bass_utils import run_bass_kernel_spmd  # noqa: E402

B = 4
N = 4096
CIN = 128
COUT = 64
P = 128                      # SBUF partitions
NJ = N // 2                  # j rows per core (adjacency half)
NJT = NJ // P                # 16 j-tiles per core
NIC = N // 512               # 8 i-chunks of 512

F32 = mybir.dt.float32
BF16 = mybir.dt.bfloat16
AF = mybir.ActivationFunctionType
ALU = mybir.AluOpType

# exp(sigmoid(x)) ~= FIT_D + FIT_A * sigmoid(FIT_B * x + FIT_C)
FIT_A = 1.71677394
FIT_B = 1.01816816
FIT_C = -0.49959447
FIT_D = 1.00040553

LAST_EXEC_NS = None
LAST_RESULTS = None


def build_nc():
    """Single-core SPMD Bass program (same program on all cores)."""
    nc = bacc.Bacc()
    adjt_d = nc.dram_tensor("adjt", [NJT // 2, P, 2 * N], BF16, kind="ExternalInput")
    xt_d = nc.dram_tensor("xt", [CIN, N], BF16, kind="ExternalInput")
    wpt_d = nc.dram_tensor("wpt", [CIN, COUT], BF16, kind="ExternalInput")
    bpcol_d = nc.dram_tensor("bpcol", [COUT, 1], F32, kind="ExternalInput")
    pkf_d = nc.dram_tensor("pkf", [P, COUT + NJT], F32, kind="ExternalInput")
    kwt_d = nc.dram_tensor("kwt", [COUT, N], BF16, kind="ExternalInput")
    kbrow_d = nc.dram_tensor("kbrow", [1, N], BF16, kind="ExternalInput")
    vwn_d = nc.dram_tensor("vwn", [P, NJT * COUT], BF16, kind="ExternalInput")
    outp_d = nc.dram_tensor("outp", [COUT, N], F32, kind="ExternalOutput")
    sgp_d = nc.dram_tensor("sgp", [1, NJT * COUT], F32, kind="ExternalOutput")

    with tile.TileContext(nc) as tc:
        with (
            tc.tile_pool(name="singles", bufs=1) as singles,
            tc.tile_pool(name="adjp", bufs=3) as adjp,
            tc.tile_pool(name="f12p", bufs=2) as f12p,
            tc.tile_pool(name="lp", bufs=3) as lp,
            tc.tile_pool(name="etp", bufs=3) as etp,
            tc.tile_pool(name="smalls", bufs=4) as smalls,
        ):
            # ---------------- parameter loads (scalar ring) --------------
            wpt_s = singles.tile([CIN, COUT], BF16)
            nc.scalar.dma_start(wpt_s, wpt_d[:, :])
            bpcol_s = singles.tile([COUT, 1], F32)
            nc.scalar.dma_start(bpcol_s, bpcol_d[:, :])
            xt_s = singles.tile([CIN, N], BF16)
            nc.scalar.dma_start(xt_s[:, 0:2048], xt_d[:, 0:2048])
            kwt_s = singles.tile([COUT, N], BF16)
            nc.scalar.dma_start(kwt_s, kwt_d[:, :])
            # prod rows 0..63 computed per chunk; row 64 carries kb, loaded
            # once (the K=65 ones-matmul adds it during the o-reduction).
            prodf = singles.tile([COUT + 1, N], BF16)
            nc.scalar.dma_start(prodf[COUT:COUT + 1, :], kbrow_d[:, :])
            nc.scalar.dma_start(xt_s[:, 2048:N], xt_d[:, 2048:N])
            # packed [128, 80] f32: proj_b broadcast (64) + vb columns (16)
            pkf_s = singles.tile([P, COUT + NJT], F32)
            nc.scalar.dma_start(pkf_s, pkf_d[:, :])
            vwn_s = singles.tile([P, NJT * COUT], BF16)
            nc.scalar.dma_start(vwn_s, vwn_d[:, :])
            bpb_s = pkf_s[:, 0:COUT]
            vbcol_s = pkf_s[:, COUT:COUT + NJT]

            ones65 = singles.tile([COUT + 1, P], BF16)
            nc.vector.memset(ones65, 1.0)
            ones1 = singles.tile([P, 1], BF16)
            nc.vector.memset(ones1, 1.0)
            cbias = singles.tile([P, 1], F32)
            nc.vector.memset(cbias, FIT_C)

            f1b = singles.tile([P, N], BF16)
            hn = singles.tile([P, NJT * COUT], F32)
            f2c = singles.tile([P, NJT], F32)
            f2cb = singles.tile([P, NJT], F32)
            g_all = singles.tile([P, NJT * COUT], BF16)

            # adjacency: two j-tiles per DMA (sync ring), 8 x 2MB
            adj_bufs = {}
            for k in range(2):
                ab = adjp.tile([P, 2 * N], BF16, tag="adj")
                nc.sync.dma_start(ab, adjt_d[k, :, :])
                adj_bufs[k] = ab

            sigmas = []  # (jt, cs, sp) pending post-sigma work

            # ---------------- pre + L/sigmoid phase (pre PSUM scoped) ------
            with (
                tc.tile_pool(name="psPreA", bufs=2, space="PSUM") as psA,
                tc.tile_pool(name="psPreB", bufs=2, space="PSUM") as psB,
            ):
                # f1 row: per chunk hT matmul, (hT+bp)*kw' STT into prodf,
                # and a K=65 ones-matmul that reduces over o, adds kb (row
                # 64), and broadcasts to all partitions; f1b copy on ACT.
                for ic in range(NIC):
                    sl = slice(ic * 512, (ic + 1) * 512)
                    psh = psA.tile([COUT, 512], F32, tag="psh")
                    nc.tensor.matmul(
                        psh, lhsT=wpt_s, rhs=xt_s[:, sl], start=True, stop=True
                    )
                    nc.vector.scalar_tensor_tensor(
                        prodf[0:COUT, sl], psh, bpcol_s, kwt_s[:, sl],
                        op0=ALU.add, op1=ALU.mult,
                    )
                    psb = psA.tile([P, 512], F32, tag="psb")
                    nc.tensor.matmul(
                        psb, lhsT=ones65, rhs=prodf[:, sl], start=True, stop=True
                    )
                    nc.scalar.copy(f1b[:, sl], psb)

                def hn_chain(t):
                    # h for j-tile t from xt slices (host rolled xt so this
                    # core's j-half is columns [0, NJ)); fused f2 via STT
                    # accum.
                    osl = slice(t * COUT, (t + 1) * COUT)
                    psn = psB.tile([P, COUT], F32, tag="psn")
                    nc.tensor.matmul(
                        psn, lhsT=xt_s[:, t * P:(t + 1) * P], rhs=wpt_s,
                        start=True, stop=True,
                    )
                    nc.vector.tensor_add(hn[:, osl], psn, bpb_s)
                    dump = smalls.tile([P, COUT], BF16, tag="dump")
                    nc.vector.scalar_tensor_tensor(
                        dump, hn[:, osl], 0.0, vwn_s[:, osl],
                        op0=ALU.add, op1=ALU.mult,
                        accum_out=f2c[:, t:t + 1],
                    )
                    nc.vector.tensor_scalar_add(
                        f2cb[:, t:t + 1], f2c[:, t:t + 1], vbcol_s[:, t:t + 1]
                    )

                hn_chain(0)
                # L + sigmoid stream; hn for tile t+1 interleaves so the
                # DVE stays one hn-chain ahead of its own L pass.
                for jt in range(NJT):
                    k, half = divmod(jt, 2)
                    if k in adj_bufs:
                        ab = adj_bufs[k]
                        if half == 1:
                            del adj_bufs[k]
                    elif half == 0:
                        ab = adjp.tile([P, 2 * N], BF16, tag="adj")
                        nc.sync.dma_start(ab, adjt_d[k, :, :])
                        adj_bufs[k] = ab
                    adj_sl = ab[:, half * N:(half + 1) * N]
                    if jt + 1 < NJT:
                        hn_chain(jt + 1)
                    # f12 = f1' + f2'[j]  (DVE tensor_scalar, 4x bf16)
                    f12 = f12p.tile([P, N], BF16, tag="f12")
                    nc.vector.tensor_scalar_add(f12, f1b, f2cb[:, jt:jt + 1])
                    # L' = f12 * adjT  (DVE tensor_tensor, 2x bf16)
                    lt = lp.tile([P, N], BF16, tag="lt")
                    nc.vector.tensor_tensor(lt, f12, adj_sl, op=ALU.mult)
                    # s' = sigmoid(L' + C), accum -> per-j partial colsum
                    sp = etp.tile([P, N], BF16, tag="sp")
                    cs = smalls.tile([P, 1], F32, tag="cs")
                    nc.scalar.activation(sp, lt, AF.Sigmoid, bias=cbias, accum_out=cs)
                    sigmas.append((jt, cs, sp))

            # ---------------- accumulation over j-tiles ----------------
            out_sb = singles.tile([COUT, N], F32)
            with tc.tile_pool(name="psMain", bufs=1, space="PSUM") as psM:
                ps_out = psM.tile([COUT, N], F32)
                for jt, cs, sp in sigmas:
                    # colsum = D*N + A*acc ; g = h/colsum ; out_T += g.T @ s'
                    t1 = smalls.tile([P, 1], F32, tag="t1")
                    nc.vector.tensor_scalar(
                        t1, cs, FIT_A, float(FIT_D * N), op0=ALU.mult, op1=ALU.add
                    )
                    rc = smalls.tile([P, 1], F32, tag="rc")
                    nc.vector.reciprocal(rc, t1)
                    gsl = slice(jt * COUT, (jt + 1) * COUT)
                    nc.vector.tensor_scalar_mul(g_all[:, gsl], hn[:, gsl], rc)
                    for c in range(NIC):
                        csl = slice(c * 512, (c + 1) * 512)
                        nc.tensor.matmul(
                            ps_out[:, csl],
                            lhsT=g_all[:, gsl],
                            rhs=sp[:, csl],
                            start=(jt == 0),
                            stop=(jt == NJT - 1),
                        )
                        if jt == NJT - 1:
                            # PSUM is not DMA-able: stage through SBUF,
                            # alternating engines, DMA per chunk.
                            if c % 2 == 0:
                                nc.vector.tensor_copy(out_sb[:, csl], ps_out[:, csl])
                            else:
                                nc.scalar.copy(out_sb[:, csl], ps_out[:, csl])
                            nc.sync.dma_start(outp_d[:, csl], out_sb[:, csl])

            # sum_j g[j,:] via ones matmul -> sgp (D-term, host combine)
            with tc.tile_pool(name="psSg", bufs=2, space="PSUM") as psS:
                sg_sb = singles.tile([1, NJT * COUT], F32)
                for hgi in range(2):
                    hsl = slice(hgi * 512, (hgi + 1) * 512)
                    ps_sg = psS.tile([1, 512], F32, tag="sg")
                    nc.tensor.matmul(
                        ps_sg, lhsT=ones1, rhs=g_all[:, hsl], start=True, stop=True
                    )
                    nc.vector.tensor_copy(sg_sb[:, hsl], ps_sg)
                nc.scalar.dma_start(sgp_d[:, :], sg_sb)

    nc.finalize()
    return nc


def _prep_in_maps(node_rep, adj_matrix, node_type, proj_W, proj_b, k_W, k_b, v_W, v_b):
    """Host-side shard prep (layout/cast/gather only, no model math)."""
    f32 = np.float32
    bf = ml_dtypes.bfloat16
    node_rep = np.asarray(node_rep, dtype=f32)
    adj = np.asarray(adj_matrix, dtype=f32)
    nt = np.asarray(node_type).astype(np.int64) % 5
    proj_W = np.asarray(proj_W, dtype=f32)
    proj_b = np.asarray(proj_b, dtype=f32)
    k_W = np.asarray(k_W, dtype=f32) * f32(FIT_B)
    k_b = np.asarray(k_b, dtype=f32) * f32(FIT_B)
    v_W = np.asarray(v_W, dtype=f32) * f32(FIT_B)
    v_b = np.asarray(v_b, dtype=f32) * f32(FIT_B)

    adjT = np.ascontiguousarray(adj.T.astype(bf))            # [j, i] bf16
    wpt = np.ascontiguousarray(proj_W.T.astype(bf))          # [CIN, COUT]
    bpcol = np.ascontiguousarray(proj_b[:, None])            # [COUT, 1]
    bpb = np.broadcast_to(proj_b[None, :], (P, COUT))
    kwt = np.ascontiguousarray(k_W[nt].T.astype(bf))         # [COUT, N]
    kbrow = np.ascontiguousarray(k_b[nt][None, :].astype(bf))  # [1, N]
    VW = v_W[nt]                                             # [N, COUT]
    vb = v_b[nt]                                             # [N]

    in_maps = []
    for core in range(8):
        b, half = divmod(core, 2)
        jsl = slice(half * NJ, (half + 1) * NJ)
        xT = np.ascontiguousarray(node_rep[b].T.astype(bf))  # [CIN, N]
        # roll the i axis so this core's j-half occupies columns [0, NJ):
        # hn then indexes xt at fixed offsets; outp is un-rolled on host.
        xTr = np.ascontiguousarray(np.roll(xT, -half * NJ, axis=1))
        kwtr = np.ascontiguousarray(np.roll(kwt, -half * NJ, axis=1))
        kbrowr = np.ascontiguousarray(np.roll(kbrow, -half * NJ, axis=1))
        # [k, p, half, i]: per double-tile k, partition p holds its two
        # j rows (j = 256k + 128*half + p) contiguously -> one 2MB DMA.
        adjr = np.ascontiguousarray(
            np.roll(adjT[jsl, :], -half * NJ, axis=1)
            .reshape(NJT // 2, 2, P, N)
            .transpose(0, 2, 1, 3)
            .reshape(NJT // 2, P, 2 * N)
        )
        vw_h = VW[jsl]                                       # [NJ, COUT]
        vwn = np.ascontiguousarray(
            vw_h.reshape(NJT, P, COUT).transpose(1, 0, 2).reshape(P, NJT * COUT)
            .astype(bf)
        )
        vbcol = vb[jsl].reshape(NJT, P).T                    # [P, NJT]
        pkf = np.ascontiguousarray(
            np.concatenate([bpb, vbcol], axis=1).astype(f32)
        )
        in_maps.append({
            "adjt": adjr,
            "xt": xTr,
            "wpt": wpt,
            "bpcol": bpcol,
            "pkf": pkf,
            "kwt": kwtr,
            "kbrow": kbrowr,
            "vwn": vwn,
        })
    return in_maps


def kernel(node_rep, adj_matrix, node_type, proj_W, proj_b, k_W, k_b, v_W, v_b):
    global LAST_EXEC_NS, LAST_RESULTS
    in_maps = _prep_in_maps(
        node_rep, adj_matrix, node_type, proj_W, proj_b, k_W, k_b, v_W, v_b
    )
    nc = build_nc()
    trace = os.environ.get("KERNEL_TRACE", "0") == "1"
    res = run_bass_kernel_spmd(nc, in_maps, core_ids=list(range(8)), trace=trace)
    LAST_EXEC_NS = res.exec_time_ns
    LAST_RESULTS = res

    out = np.empty((B, N, COUT), dtype=np.float32)
    for b in range(B):
        m = None
        sg = None
        for half in range(2):
            r = res.results[2 * b + half]
            mp = np.asarray(r["outp"], dtype=np.float32)          # [COUT, N]
            mp = np.roll(mp, half * NJ, axis=1)  # un-roll the i axis
            sp = np.asarray(r["sgp"], dtype=np.float32).reshape(NJT, COUT).sum(axis=0)
            m = mp if m is None else m + mp
            sg = sp if sg is None else sg + sp
        out[b] = FIT_A * m.T + FIT_D * sg[None, :]
    return out


# revision 15
# speedup vs baseline: 1.3394x; 1.1146x over previous
"""HGAT message-passing kernel for Trainium2 (8 NeuronCores, SPMD).

Reference computation (B=4, N=4096, C_IN=128, C_OUT=64):
    h   = node_rep @ proj_W.T + proj_b                    # [B,N,64]
    f1  = rowsum(h * k_W[node_type]) + k_b[node_type]     # [B,N]
    f2  = rowsum(h * v_W[node_type]) + v_b[node_type]     # [B,N]
    L   = adj[i,j] * (f1[i] + f2[j])
    u   = sigmoid(L) - 0.5
    P   = softmax(u, axis=i)      # normalized over rows i, per column j
    out = P @ h                   # contract over j

Key algebra used on device:
  * softmax-over-i / contract-over-j means out = E @ (h / colsum) with
    E[i,j] = exp(sigmoid(L)) and colsum[j] = sum_i E[i,j]; the -0.5 and the
    softmax max-subtraction cancel in the ratio.
  * exp(sigmoid(x)) ~= D + A*sigmoid(B*x + C) with max rel err 4.1e-4, so
    ONE ACT pass (Sigmoid, accum_out -> colsum) gives s'; the B scale folds
    into host-prescaled k/v params, C is the ACT bias, A and the rank-1
    D-term fold out on the host combine.
  * f12 = f1 + f2[j] is a DVE tensor_scalar (4x bf16); L' = f12*adjT a DVE
    tensor_tensor (2x bf16). (A fused scalar_tensor_tensor runs at 1x on
    HW — slower than the pair.)
  * f2 rowsum fuses into one small STT with accum_out.
  * final matmul is transposed (out_T[o,i] = g.T @ s'), bf16 both sides;
    the D-term needs only sum_j g[j,:], computed on-device by a ones
    matmul (sgp output), so g never travels to the host.
  * adjacency travels as bf16 (tolerance 2e-2; quantization adds ~1e-4).

Sharding: core c handles batch b=c//2 and j-half h=c%2 (rows of adj.T).
The i axis is rolled per-core so the core's own j columns sit first in
xt — hn then reads fixed xt slices; the host un-rolls outp at the end.

DMA issue costs ~600ns of queue time each, so transfers are batched:
xt in 2, kwt/kbrow/params in 1 each (scalar ring), adjacency as 8 x 2MB
double-tile loads (sync ring). Output staging copies alternate DVE/ACT.
"""

import os
import sys

import numpy as np

sys.path.insert(0, "/opt/trn_rl_repo")

import ml_dtypes  # noqa: E402

import concourse.tile as tile  # noqa: E402
from concourse import bacc  # noqa: E402
from concourse import mybir  # noqa: E402
from concourse.bass_utils import run_bass_kernel_spmd  # noqa: E402

B = 4
N = 4096
CIN = 128
COUT = 64
P = 128                      # SBUF partitions
NJ = N // 2                  # j rows per core (adjacency half)
NJT = NJ // P                # 16 j-tiles per core
NIC = N // 512               # 8 i-chunks of 512

F32 = mybir.dt.float32
BF16 = mybir.dt.bfloat16
AF = mybir.ActivationFunctionType
ALU = mybir.AluOpType

# exp(sigmoid(x)) ~= FIT_D + FIT_A * sigmoid(FIT_B * x + FIT_C)
FIT_A = 1.71677394
FIT_B = 1.01816816
FIT_C = -0.49959447
FIT_D = 1.00040553

LAST_EXEC_NS = None
LAST_RESULTS = None


def build_nc():
    """Single-core SPMD Bass program (same program on all cores)."""
    nc = bacc.Bacc()
    adjt_d = nc.dram_tensor("adjt", [NJT // 2, P, 2 * N], BF16, kind="ExternalInput")
    xt_d = nc.dram_tensor("xt", [CIN, N], BF16, kind="ExternalInput")
    wpt_d = nc.dram_tensor("wpt", [CIN, COUT], BF16, kind="ExternalInput")
    bpcol_d = nc.dram_tensor("bpcol", [COUT, 1], F32, kind="ExternalInput")
    pkf_d = nc.dram_tensor("pkf", [P, COUT + NJT], F32, kind="ExternalInput")
    kwt_d = nc.dram_tensor("kwt", [COUT, N], BF16, kind="ExternalInput")
    kbrow_d = nc.dram_tensor("kbrow", [1, N], BF16, kind="ExternalInput")
    vwn_d = nc.dram_tensor("vwn", [P, NJT * COUT], BF16, kind="ExternalInput")
    outp_d = nc.dram_tensor("outp", [COUT, N], F32, kind="ExternalOutput")
    sgp_d = nc.dram_tensor("sgp", [1, NJT * COUT], F32, kind="ExternalOutput")

    with tile.TileContext(nc) as tc:
        with (
            tc.tile_pool(name="singles", bufs=1) as singles,
            tc.tile_pool(name="adjp", bufs=4) as adjp,
            tc.tile_pool(name="f12p", bufs=2) as f12p,
            tc.tile_pool(name="lp", bufs=2) as lp,
            tc.tile_pool(name="etp", bufs=3) as etp,
            tc.tile_pool(name="smalls", bufs=4) as smalls,
        ):
            # ------------- input loads (one ring, priority order) --------
            # One HBM pipe: order by need. xt/kwt feed the f1 critical
            # path; adjacency (16MB) goes behind them so it can't crowd
            # them out of the shared SDMA engines.
            wpt_s = singles.tile([CIN, COUT], BF16)
            nc.sync.dma_start(wpt_s, wpt_d[:, :])
            bpcol_s = singles.tile([COUT, 1], F32)
            nc.sync.dma_start(bpcol_s, bpcol_d[:, :])
            xt_s = singles.tile([CIN, N], BF16)
            nc.sync.dma_start(xt_s[:, 0:2048], xt_d[:, 0:2048])
            kwt_s = singles.tile([COUT, N], BF16)
            nc.sync.dma_start(kwt_s, kwt_d[:, :])
            # prod rows 0..63 computed per chunk; row 64 carries kb, loaded
            # once (the K=65 ones-matmul adds it during the o-reduction).
            prodf = singles.tile([COUT + 1, N], BF16)
            nc.sync.dma_start(prodf[COUT:COUT + 1, :], kbrow_d[:, :])
            nc.sync.dma_start(xt_s[:, 2048:N], xt_d[:, 2048:N])
            # packed [128, 80] f32: proj_b broadcast (64) + vb columns (16)
            pkf_s = singles.tile([P, COUT + NJT], F32)
            nc.sync.dma_start(pkf_s, pkf_d[:, :])
            vwn_s = singles.tile([P, NJT * COUT], BF16)
            nc.sync.dma_start(vwn_s, vwn_d[:, :])
            bpb_s = pkf_s[:, 0:COUT]
            vbcol_s = pkf_s[:, COUT:COUT + NJT]

            ones65 = singles.tile([COUT + 1, P], BF16)
            nc.vector.memset(ones65, 1.0)
            ones1 = singles.tile([P, 1], BF16)
            nc.vector.memset(ones1, 1.0)
            cbias = singles.tile([P, 1], F32)
            nc.vector.memset(cbias, FIT_C)

            f1b = singles.tile([P, N], BF16)
            hn = singles.tile([P, NJT * COUT], F32)
            f2c = singles.tile([P, NJT], F32)
            f2cb = singles.tile([P, NJT], F32)
            g_all = singles.tile([P, NJT * COUT], BF16)

            # adjacency: two j-tiles per DMA (sync ring), 8 x 2MB; 4 bufs
            # = 8 j-tiles (~28us) of prefetch to ride out refill latency.
            adj_bufs = {}
            for k in range(4):
                ab = adjp.tile([P, 2 * N], BF16, tag="adj")
                nc.sync.dma_start(ab, adjt_d[k, :, :])
                adj_bufs[k] = ab

            sigmas = []  # (jt, cs, sp) pending post-sigma work

            # ---------------- pre + L/sigmoid phase (pre PSUM scoped) ------
            with (
                tc.tile_pool(name="psPreA", bufs=2, space="PSUM") as psA,
                tc.tile_pool(name="psPreB", bufs=2, space="PSUM") as psB,
            ):
                # f1 row: per chunk hT matmul, (hT+bp)*kw' STT into prodf,
                # and a K=65 ones-matmul that reduces over o, adds kb (row
                # 64), and broadcasts to all partitions; f1b copy on ACT.
                for ic in range(NIC):
                    sl = slice(ic * 512, (ic + 1) * 512)
                    psh = psA.tile([COUT, 512], F32, tag="psh")
                    nc.tensor.matmul(
                        psh, lhsT=wpt_s, rhs=xt_s[:, sl], start=True, stop=True
                    )
                    nc.vector.scalar_tensor_tensor(
                        prodf[0:COUT, sl], psh, bpcol_s, kwt_s[:, sl],
                        op0=ALU.add, op1=ALU.mult,
                    )
                    psb = psA.tile([P, 512], F32, tag="psb")
                    nc.tensor.matmul(
                        psb, lhsT=ones65, rhs=prodf[:, sl], start=True, stop=True
                    )
                    nc.scalar.copy(f1b[:, sl], psb)

                def hn_chain(t):
                    # h for j-tile t from xt slices (host rolled xt so this
                    # core's j-half is columns [0, NJ)); fused f2 via STT
                    # accum.
                    osl = slice(t * COUT, (t + 1) * COUT)
                    psn = psB.tile([P, COUT], F32, tag="psn")
                    nc.tensor.matmul(
                        psn, lhsT=xt_s[:, t * P:(t + 1) * P], rhs=wpt_s,
                        start=True, stop=True,
                    )
                    nc.vector.tensor_add(hn[:, osl], psn, bpb_s)
                    dump = smalls.tile([P, COUT], BF16, tag="dump")
                    nc.vector.scalar_tensor_tensor(
                        dump, hn[:, osl], 0.0, vwn_s[:, osl],
                        op0=ALU.add, op1=ALU.mult,
                        accum_out=f2c[:, t:t + 1],
                    )
                    nc.vector.tensor_scalar_add(
                        f2cb[:, t:t + 1], f2c[:, t:t + 1], vbcol_s[:, t:t + 1]
                    )

                hn_chain(0)
                # L + sigmoid stream; hn for tile t+1 interleaves so the
                # DVE stays one hn-chain ahead of its own L pass. The first
                # and last tiles run as 2048-column halves to shrink the
                # pipeline fill (sigma starts on half a tile) and drain
                # (final matmuls start on half a tile).
                for jt in range(NJT):
                    k, half = divmod(jt, 2)
                    if k in adj_bufs:
                        ab = adj_bufs[k]
                        if half == 1:
                            del adj_bufs[k]
                    elif half == 0:
                        ab = adjp.tile([P, 2 * N], BF16, tag="adj")
                        nc.sync.dma_start(ab, adjt_d[k, :, :])
                        adj_bufs[k] = ab
                    adj_sl = ab[:, half * N:(half + 1) * N]
                    if jt + 1 < NJT:
                        hn_chain(jt + 1)
                    split = jt == 0 or jt == NJT - 1
                    f12 = f12p.tile([P, N], BF16, tag="f12")
                    lt = lp.tile([P, N], BF16, tag="lt")
                    sp = etp.tile([P, N], BF16, tag="sp")
                    if split:
                        cs2 = smalls.tile([P, 2], F32, tag="cs2")
                        for hh in range(2):
                            hsl = slice(hh * 2048, (hh + 1) * 2048)
                            nc.vector.tensor_scalar_add(
                                f12[:, hsl], f1b[:, hsl], f2cb[:, jt:jt + 1]
                            )
                            nc.vector.tensor_tensor(
                                lt[:, hsl], f12[:, hsl], adj_sl[:, hsl],
                                op=ALU.mult,
                            )
                            nc.scalar.activation(
                                sp[:, hsl], lt[:, hsl], AF.Sigmoid, bias=cbias,
                                accum_out=cs2[:, hh:hh + 1],
                            )
                        cs = smalls.tile([P, 1], F32, tag="cs")
                        nc.vector.tensor_add(cs, cs2[:, 0:1], cs2[:, 1:2])
                    else:
                        # f12 = f1' + f2'[j] (4x bf16); L' = f12*adjT (2x)
                        nc.vector.tensor_scalar_add(f12, f1b, f2cb[:, jt:jt + 1])
                        nc.vector.tensor_tensor(lt, f12, adj_sl, op=ALU.mult)
                        # s' = sigmoid(L'+C), accum -> per-j partial colsum
                        sp_cs = smalls.tile([P, 1], F32, tag="cs")
                        nc.scalar.activation(
                            sp, lt, AF.Sigmoid, bias=cbias, accum_out=sp_cs
                        )
                        cs = sp_cs
                    sigmas.append((jt, cs, sp))

            # ---------------- accumulation over j-tiles ----------------
            out_sb = singles.tile([COUT, N], F32)
            with tc.tile_pool(name="psMain", bufs=1, space="PSUM") as psM:
                ps_out = psM.tile([COUT, N], F32)
                for jt, cs, sp in sigmas:
                    # colsum = D*N + A*acc ; g = h/colsum ; out_T += g.T @ s'
                    t1 = smalls.tile([P, 1], F32, tag="t1")
                    nc.vector.tensor_scalar(
                        t1, cs, FIT_A, float(FIT_D * N), op0=ALU.mult, op1=ALU.add
                    )
                    rc = smalls.tile([P, 1], F32, tag="rc")
                    nc.vector.reciprocal(rc, t1)
                    gsl = slice(jt * COUT, (jt + 1) * COUT)
                    nc.vector.tensor_scalar_mul(g_all[:, gsl], hn[:, gsl], rc)
                    for c in range(NIC):
                        csl = slice(c * 512, (c + 1) * 512)
                        nc.tensor.matmul(
                            ps_out[:, csl],
                            lhsT=g_all[:, gsl],
                            rhs=sp[:, csl],
                            start=(jt == 0),
                            stop=(jt == NJT - 1),
                        )
                        if jt == NJT - 1:
                            # PSUM is not DMA-able: stage through SBUF,
                            # alternating engines, DMA per chunk.
                            if c % 2 == 0:
                                nc.vector.tensor_copy(out_sb[:, csl], ps_out[:, csl])
                            else:
                                nc.scalar.copy(out_sb[:, csl], ps_out[:, csl])
                            nc.sync.dma_start(outp_d[:, csl], out_sb[:, csl])

            # sum_j g[j,:] via ones matmul -> sgp (D-term, host combine)
            with tc.tile_pool(name="psSg", bufs=2, space="PSUM") as psS:
                sg_sb = singles.tile([1, NJT * COUT], F32)
                for hgi in range(2):
                    hsl = slice(hgi * 512, (hgi + 1) * 512)
                    ps_sg = psS.tile([1, 512], F32, tag="sg")
                    nc.tensor.matmul(
                        ps_sg, lhsT=ones1, rhs=g_all[:, hsl], start=True, stop=True
                    )
                    nc.vector.tensor_copy(sg_sb[:, hsl], ps_sg)
                nc.scalar.dma_start(sgp_d[:, :], sg_sb)

    nc.finalize()
    return nc


def _prep_in_maps(node_rep, adj_matrix, node_type, proj_W, proj_b, k_W, k_b, v_W, v_b):
    """Host-side shard prep (layout/cast/gather only, no model math)."""
    f32 = np.float32
    bf = ml_dtypes.bfloat16
    node_rep = np.asarray(node_rep, dtype=f32)
    adj = np.asarray(adj_matrix, dtype=f32)
    nt = np.asarray(node_type).astype(np.int64) % 5
    proj_W = np.asarray(proj_W, dtype=f32)
    proj_b = np.asarray(proj_b, dtype=f32)
    k_W = np.asarray(k_W, dtype=f32) * f32(FIT_B)
    k_b = np.asarray(k_b, dtype=f32) * f32(FIT_B)
    v_W = np.asarray(v_W, dtype=f32) * f32(FIT_B)
    v_b = np.asarray(v_b, dtype=f32) * f32(FIT_B)

    adjT = np.ascontiguousarray(adj.T.astype(bf))            # [j, i] bf16
    wpt = np.ascontiguousarray(proj_W.T.astype(bf))          # [CIN, COUT]
    bpcol = np.ascontiguousarray(proj_b[:, None])            # [COUT, 1]
    bpb = np.broadcast_to(proj_b[None, :], (P, COUT))
    kwt = np.ascontiguousarray(k_W[nt].T.astype(bf))         # [COUT, N]
    kbrow = np.ascontiguousarray(k_b[nt][None, :].astype(bf))  # [1, N]
    VW = v_W[nt]                                             # [N, COUT]
    vb = v_b[nt]                                             # [N]

    in_maps = []
    for core in range(8):
        b, half = divmod(core, 2)
        jsl = slice(half * NJ, (half + 1) * NJ)
        xT = np.ascontiguousarray(node_rep[b].T.astype(bf))  # [CIN, N]
        # roll the i axis so this core's j-half occupies columns [0, NJ):
        # hn then indexes xt at fixed offsets; outp is un-rolled on host.
        xTr = np.ascontiguousarray(np.roll(xT, -half * NJ, axis=1))
        kwtr = np.ascontiguousarray(np.roll(kwt, -half * NJ, axis=1))
        kbrowr = np.ascontiguousarray(np.roll(kbrow, -half * NJ, axis=1))
        # [k, p, half, i]: per double-tile k, partition p holds its two
        # j rows (j = 256k + 128*half + p) contiguously -> one 2MB DMA.
        adjr = np.ascontiguousarray(
            np.roll(adjT[jsl, :], -half * NJ, axis=1)
            .reshape(NJT // 2, 2, P, N)
            .transpose(0, 2, 1, 3)
            .reshape(NJT // 2, P, 2 * N)
        )
        vw_h = VW[jsl]                                       # [NJ, COUT]
        vwn = np.ascontiguousarray(
            vw_h.reshape(NJT, P, COUT).transpose(1, 0, 2).reshape(P, NJT * COUT)
            .astype(bf)
        )
        vbcol = vb[jsl].reshape(NJT, P).T                    # [P, NJT]
        pkf = np.ascontiguousarray(
            np.concatenate([bpb, vbcol], axis=1).astype(f32)
        )
        in_maps.append({
            "adjt": adjr,
            "xt": xTr,
            "wpt": wpt,
            "bpcol": bpcol,
            "pkf": pkf,
            "kwt": kwtr,
            "kbrow": kbrowr,
            "vwn": vwn,
        })
    return in_maps


def kernel(node_rep, adj_matrix, node_type, proj_W, proj_b, k_W, k_b, v_W, v_b):
    global LAST_EXEC_NS, LAST_RESULTS
    in_maps = _prep_in_maps(
        node_rep, adj_matrix, node_type, proj_W, proj_b, k_W, k_b, v_W, v_b
    )
    nc = build_nc()
    trace = os.environ.get("KERNEL_TRACE", "0") == "1"
    res = run_bass_kernel_spmd(nc, in_maps, core_ids=list(range(8)), trace=trace)
    LAST_EXEC_NS = res.exec_time_ns
    LAST_RESULTS = res

    out = np.empty((B, N, COUT), dtype=np.float32)
    for b in range(B):
        m = None
        sg = None
        for half in range(2):
            r = res.results[2 * b + half]
            mp = np.asarray(r["outp"], dtype=np.float32)          # [COUT, N]
            mp = np.roll(mp, half * NJ, axis=1)  # un-roll the i axis
            sp = np.asarray(r["sgp"], dtype=np.float32).reshape(NJT, COUT).sum(axis=0)
            m = mp if m is None else m + mp
            sg = sp if sg is None else sg + sp
        out[b] = FIT_A * m.T + FIT_D * sg[None, :]
    return out


# revision 17
# speedup vs baseline: 1.4464x; 1.0799x over previous
"""HGAT message-passing kernel for Trainium2 (8 NeuronCores, SPMD).

Reference computation (B=4, N=4096, C_IN=128, C_OUT=64):
    h   = node_rep @ proj_W.T + proj_b                    # [B,N,64]
    f1  = rowsum(h * k_W[node_type]) + k_b[node_type]     # [B,N]
    f2  = rowsum(h * v_W[node_type]) + v_b[node_type]     # [B,N]
    L   = adj[i,j] * (f1[i] + f2[j])
    u   = sigmoid(L) - 0.5
    P   = softmax(u, axis=i)      # normalized over rows i, per column j
    out = P @ h                   # contract over j

Key algebra used on device:
  * softmax-over-i / contract-over-j means out = E @ (h / colsum) with
    E[i,j] = exp(sigmoid(L)) and colsum[j] = sum_i E[i,j]; the -0.5 and the
    softmax max-subtraction cancel in the ratio.
  * exp(sigmoid(x)) ~= D + A*sigmoid(B*x + C) with max rel err 4.1e-4, so
    ONE ACT pass (Sigmoid, accum_out -> colsum) gives s'; the B scale folds
    into host-prescaled k/v params, C is the ACT bias, A and the rank-1
    D-term fold out on the host combine.
  * f12 = f1 + f2[j] is a DVE tensor_scalar (4x bf16); L' = f12*adjT a DVE
    tensor_tensor (2x bf16). (A fused scalar_tensor_tensor runs at 1x on
    HW — slower than the pair.)
  * f2 rowsum fuses into one small STT with accum_out.
  * final matmul is transposed (out_T[o,i] = g.T @ s'), bf16 both sides;
    the D-term needs only sum_j g[j,:], computed on-device by a ones
    matmul (sgp output), so g never travels to the host.
  * adjacency travels as bf16 (tolerance 2e-2; quantization adds ~1e-4).

Sharding: core c handles batch b=c//2 and j-half h=c%2 (rows of adj.T).
The i axis is rolled per-core so the core's own j columns sit first in
xt — hn then reads fixed xt slices; the host un-rolls outp at the end.

DMA issue costs ~600ns of queue time each, so transfers are batched:
xt in 2, kwt/kbrow/params in 1 each (scalar ring), adjacency as 8 x 2MB
double-tile loads (sync ring). Output staging copies alternate DVE/ACT.
"""

import os
import sys

import numpy as np

sys.path.insert(0, "/opt/trn_rl_repo")

import ml_dtypes  # noqa: E402

import concourse.tile as tile  # noqa: E402
from concourse import bacc  # noqa: E402
from concourse import mybir  # noqa: E402
from concourse.bass_utils import run_bass_kernel_spmd  # noqa: E402

B = 4
N = 4096
CIN = 128
COUT = 64
P = 128                      # SBUF partitions
NJ = N // 2                  # j rows per core (adjacency half)
NJT = NJ // P                # 16 j-tiles per core
NIC = N // 512               # 8 i-chunks of 512

F32 = mybir.dt.float32
BF16 = mybir.dt.bfloat16
AF = mybir.ActivationFunctionType
ALU = mybir.AluOpType

# exp(sigmoid(x)) ~= FIT_D + FIT_A * sigmoid(FIT_B * x + FIT_C)
FIT_A = 1.71677394
FIT_B = 1.01816816
FIT_C = -0.49959447
FIT_D = 1.00040553

LAST_EXEC_NS = None
LAST_RESULTS = None


def build_nc():
    """Single-core SPMD Bass program (same program on all cores)."""
    nc = bacc.Bacc()
    adjt_d = nc.dram_tensor("adjt", [NJT // 2, P, 2 * N], BF16, kind="ExternalInput")
    xt_d = nc.dram_tensor("xt", [CIN, N], BF16, kind="ExternalInput")
    wpt_d = nc.dram_tensor("wpt", [CIN, COUT], BF16, kind="ExternalInput")
    bpcol_d = nc.dram_tensor("bpcol", [COUT, 1], F32, kind="ExternalInput")
    pkf_d = nc.dram_tensor("pkf", [P, COUT + NJT], F32, kind="ExternalInput")
    kwt_d = nc.dram_tensor("kwt", [COUT, N], BF16, kind="ExternalInput")
    kbrow_d = nc.dram_tensor("kbrow", [1, N], BF16, kind="ExternalInput")
    vwn_d = nc.dram_tensor("vwn", [P, NJT * COUT], BF16, kind="ExternalInput")
    outp_d = nc.dram_tensor("outp", [COUT, N], F32, kind="ExternalOutput")
    sgp_d = nc.dram_tensor("sgp", [1, NJT * COUT], F32, kind="ExternalOutput")

    with tile.TileContext(nc) as tc:
        with (
            tc.tile_pool(name="singles", bufs=1) as singles,
            tc.tile_pool(name="adjp", bufs=4) as adjp,
            tc.tile_pool(name="f12p", bufs=2) as f12p,
            tc.tile_pool(name="lp", bufs=2) as lp,
            tc.tile_pool(name="etp", bufs=4) as etp,
            tc.tile_pool(name="smalls", bufs=4) as smalls,
        ):
            # ------------- input loads (one ring, priority order) --------
            # One HBM pipe: order by need. xt/kwt feed the f1 critical
            # path; adjacency (16MB) goes behind them so it can't crowd
            # them out of the shared SDMA engines.
            wpt_s = singles.tile([CIN, COUT], BF16)
            nc.sync.dma_start(wpt_s, wpt_d[:, :])
            bpcol_s = singles.tile([COUT, 1], F32)
            nc.sync.dma_start(bpcol_s, bpcol_d[:, :])
            xt_s = singles.tile([CIN, N], BF16)
            nc.sync.dma_start(xt_s[:, 0:2048], xt_d[:, 0:2048])
            kwt_s = singles.tile([COUT, N], BF16)
            nc.sync.dma_start(kwt_s, kwt_d[:, :])
            # prod rows 0..63 computed per chunk; row 64 carries kb, loaded
            # once (the K=65 ones-matmul adds it during the o-reduction).
            prodf = singles.tile([COUT + 1, N], BF16)
            nc.sync.dma_start(prodf[COUT:COUT + 1, :], kbrow_d[:, :])
            nc.sync.dma_start(xt_s[:, 2048:N], xt_d[:, 2048:N])
            # packed [128, 80] f32: proj_b broadcast (64) + vb columns (16)
            pkf_s = singles.tile([P, COUT + NJT], F32)
            nc.sync.dma_start(pkf_s, pkf_d[:, :])
            vwn_s = singles.tile([P, NJT * COUT], BF16)
            nc.sync.dma_start(vwn_s, vwn_d[:, :])
            bpb_s = pkf_s[:, 0:COUT]
            vbcol_s = pkf_s[:, COUT:COUT + NJT]

            ones65 = singles.tile([COUT + 1, P], BF16)
            nc.vector.memset(ones65, 1.0)
            ones1 = singles.tile([P, 1], BF16)
            nc.vector.memset(ones1, 1.0)
            cbias = singles.tile([P, 1], F32)
            nc.vector.memset(cbias, FIT_C)

            f1b = singles.tile([P, N], BF16)
            hn = singles.tile([P, NJT * COUT], F32)
            f2c = singles.tile([P, NJT], F32)
            f2cb = singles.tile([P, NJT], F32)
            g_all = singles.tile([P, NJT * COUT], BF16)

            # adjacency: two j-tiles per DMA (sync ring), 8 x 2MB; 4 bufs
            # = 8 j-tiles (~28us) of prefetch to ride out refill latency.
            adj_bufs = {}
            for k in range(4):
                ab = adjp.tile([P, 2 * N], BF16, tag="adj")
                nc.sync.dma_start(ab, adjt_d[k, :, :])
                adj_bufs[k] = ab

            def f2part(t):
                # fused f2 rowsum via STT accum + vb add (no PSUM)
                osl = slice(t * COUT, (t + 1) * COUT)
                dump = smalls.tile([P, COUT], BF16, tag="dump")
                nc.vector.scalar_tensor_tensor(
                    dump, hn[:, osl], 0.0, vwn_s[:, osl],
                    op0=ALU.add, op1=ALU.mult,
                    accum_out=f2c[:, t:t + 1],
                )
                nc.vector.tensor_scalar_add(
                    f2cb[:, t:t + 1], f2c[:, t:t + 1], vbcol_s[:, t:t + 1]
                )

            def lt_sigma(jt, adj_sl, halves):
                # f12 = f1' + f2'[j] (4x bf16); L' = f12*adjT (2x bf16);
                # s' = sigmoid(L' + C) with accum -> per-j partial colsum.
                # halves=True runs 2048-col halves (pipeline fill/drain).
                f12 = f12p.tile([P, N], BF16, tag="f12")
                lt = lp.tile([P, N], BF16, tag="lt")
                sp = etp.tile([P, N], BF16, tag="sp")
                if halves:
                    cs2 = smalls.tile([P, 2], F32, tag="cs2")
                    for hh in range(2):
                        hsl = slice(hh * 2048, (hh + 1) * 2048)
                        nc.vector.tensor_scalar_add(
                            f12[:, hsl], f1b[:, hsl], f2cb[:, jt:jt + 1]
                        )
                        nc.vector.tensor_tensor(
                            lt[:, hsl], f12[:, hsl], adj_sl[:, hsl], op=ALU.mult
                        )
                        nc.scalar.activation(
                            sp[:, hsl], lt[:, hsl], AF.Sigmoid, bias=cbias,
                            accum_out=cs2[:, hh:hh + 1],
                        )
                    cs = smalls.tile([P, 1], F32, tag="cs")
                    nc.vector.tensor_add(cs, cs2[:, 0:1], cs2[:, 1:2])
                else:
                    nc.vector.tensor_scalar_add(f12, f1b, f2cb[:, jt:jt + 1])
                    nc.vector.tensor_tensor(lt, f12, adj_sl, op=ALU.mult)
                    cs = smalls.tile([P, 1], F32, tag="cs")
                    nc.scalar.activation(
                        sp, lt, AF.Sigmoid, bias=cbias, accum_out=cs
                    )
                return cs, sp

            def adj_slice(jt):
                k, half = divmod(jt, 2)
                if k in adj_bufs:
                    ab = adj_bufs[k]
                    if half == 1:
                        del adj_bufs[k]
                elif half == 0:
                    ab = adjp.tile([P, 2 * N], BF16, tag="adj")
                    nc.sync.dma_start(ab, adjt_d[k, :, :])
                    adj_bufs[k] = ab
                else:
                    raise AssertionError
                return ab[:, half * N:(half + 1) * N]

            # ---------------- pre-phase (PSUM pools scoped) ----------------
            # f1 row: per chunk hT matmul, (hT+bp)*kw' STT into prodf, and
            # a K=65 ones-matmul that reduces over o, adds kb (row 64), and
            # broadcasts to all partitions; f1b copies on the (still idle)
            # ACT. Tile 0's first sigmoid half is wedged in after f1b's
            # first half so the ACT stream starts ~8us earlier. hn = h for
            # this core's j columns (host rolled xt so they sit first).
            sigmas = []

            def hn_part(t):
                osl = slice(t * COUT, (t + 1) * COUT)
                psn = psB.tile([P, COUT], F32, tag="psn")
                nc.tensor.matmul(
                    psn, lhsT=xt_s[:, t * P:(t + 1) * P], rhs=wpt_s,
                    start=True, stop=True,
                )
                nc.vector.tensor_add(hn[:, osl], psn, bpb_s)

            def f1_chunk(ic):
                sl = slice(ic * 512, (ic + 1) * 512)
                psh = psA.tile([COUT, 512], F32, tag="psh")
                nc.tensor.matmul(
                    psh, lhsT=wpt_s, rhs=xt_s[:, sl], start=True, stop=True
                )
                nc.vector.scalar_tensor_tensor(
                    prodf[0:COUT, sl], psh, bpcol_s, kwt_s[:, sl],
                    op0=ALU.add, op1=ALU.mult,
                )
                psb = psA.tile([P, 512], F32, tag="psb")
                nc.tensor.matmul(
                    psb, lhsT=ones65, rhs=prodf[:, sl], start=True, stop=True
                )
                nc.scalar.copy(f1b[:, sl], psb)

            with (
                tc.tile_pool(name="psPreA", bufs=2, space="PSUM") as psA,
                tc.tile_pool(name="psPreB", bufs=2, space="PSUM") as psB,
            ):
                for t in range(3):
                    hn_part(t)
                    f2part(t)
                for ic in range(4):
                    f1_chunk(ic)
                # first sigmoid half needs only f1b[:, :2048] + f2cb[0]
                sp0 = etp.tile([P, N], BF16, tag="sp")
                f12_0 = f12p.tile([P, N], BF16, tag="f12")
                lt0 = lp.tile([P, N], BF16, tag="lt")
                cs2_0 = smalls.tile([P, 2], F32, tag="cs2")
                adj0_sl = adj_slice(0)
                for hh in range(2):
                    if hh == 1:
                        for ic in range(4, NIC):
                            f1_chunk(ic)
                    hsl = slice(hh * 2048, (hh + 1) * 2048)
                    nc.vector.tensor_scalar_add(
                        f12_0[:, hsl], f1b[:, hsl], f2cb[:, 0:1]
                    )
                    nc.vector.tensor_tensor(
                        lt0[:, hsl], f12_0[:, hsl], adj0_sl[:, hsl], op=ALU.mult
                    )
                    nc.scalar.activation(
                        sp0[:, hsl], lt0[:, hsl], AF.Sigmoid, bias=cbias,
                        accum_out=cs2_0[:, hh:hh + 1],
                    )
                cs0 = smalls.tile([P, 1], F32, tag="cs")
                nc.vector.tensor_add(cs0, cs2_0[:, 0:1], cs2_0[:, 1:2])
                sigmas.append((0, cs0, sp0))
                for t in range(3, NJT):
                    hn_part(t)

            # ---------------- main loop + accumulation ----------------
            out_sb = singles.tile([COUT, N], F32)
            with tc.tile_pool(name="psMain", bufs=1, space="PSUM") as psM:
                ps_out = psM.tile([COUT, N], F32)

                def post_sigma(jt, cs, sp):
                    # colsum = D*N + A*acc ; g = h/colsum ; out_T += g.T @ s'
                    t1 = smalls.tile([P, 1], F32, tag="t1")
                    nc.vector.tensor_scalar(
                        t1, cs, FIT_A, float(FIT_D * N), op0=ALU.mult, op1=ALU.add
                    )
                    rc = smalls.tile([P, 1], F32, tag="rc")
                    nc.vector.reciprocal(rc, t1)
                    gsl = slice(jt * COUT, (jt + 1) * COUT)
                    nc.vector.tensor_scalar_mul(g_all[:, gsl], hn[:, gsl], rc)
                    for c in range(NIC):
                        csl = slice(c * 512, (c + 1) * 512)
                        nc.tensor.matmul(
                            ps_out[:, csl],
                            lhsT=g_all[:, gsl],
                            rhs=sp[:, csl],
                            start=(jt == 0),
                            stop=(jt == NJT - 1),
                        )
                        if jt == NJT - 1:
                            # PSUM is not DMA-able: stage through SBUF,
                            # alternating engines, DMA per chunk.
                            if c % 2 == 0:
                                nc.vector.tensor_copy(out_sb[:, csl], ps_out[:, csl])
                            else:
                                nc.scalar.copy(out_sb[:, csl], ps_out[:, csl])
                            nc.sync.dma_start(outp_d[:, csl], out_sb[:, csl])

                pend = sigmas[0]
                for jt in range(1, NJT):
                    if jt + 2 < NJT:
                        f2part(jt + 2)
                    cs, sp = lt_sigma(jt, adj_slice(jt), halves=(jt == NJT - 1))
                    post_sigma(*pend)
                    pend = (jt, cs, sp)
                post_sigma(*pend)

            # sum_j g[j,:] via ones matmul -> sgp (D-term, host combine)
            with tc.tile_pool(name="psSg", bufs=2, space="PSUM") as psS:
                sg_sb = singles.tile([1, NJT * COUT], F32)
                for hgi in range(2):
                    hsl = slice(hgi * 512, (hgi + 1) * 512)
                    ps_sg = psS.tile([1, 512], F32, tag="sg")
                    nc.tensor.matmul(
                        ps_sg, lhsT=ones1, rhs=g_all[:, hsl], start=True, stop=True
                    )
                    nc.vector.tensor_copy(sg_sb[:, hsl], ps_sg)
                nc.scalar.dma_start(sgp_d[:, :], sg_sb)

    nc.finalize()
    return nc


def _prep_in_maps(node_rep, adj_matrix, node_type, proj_W, proj_b, k_W, k_b, v_W, v_b):
    """Host-side shard prep (layout/cast/gather only, no model math)."""
    f32 = np.float32
    bf = ml_dtypes.bfloat16
    node_rep = np.asarray(node_rep, dtype=f32)
    adj = np.asarray(adj_matrix, dtype=f32)
    nt = np.asarray(node_type).astype(np.int64) % 5
    proj_W = np.asarray(proj_W, dtype=f32)
    proj_b = np.asarray(proj_b, dtype=f32)
    k_W = np.asarray(k_W, dtype=f32) * f32(FIT_B)
    k_b = np.asarray(k_b, dtype=f32) * f32(FIT_B)
    v_W = np.asarray(v_W, dtype=f32) * f32(FIT_B)
    v_b = np.asarray(v_b, dtype=f32) * f32(FIT_B)

    adjT = np.ascontiguousarray(adj.T.astype(bf))            # [j, i] bf16
    wpt = np.ascontiguousarray(proj_W.T.astype(bf))          # [CIN, COUT]
    bpcol = np.ascontiguousarray(proj_b[:, None])            # [COUT, 1]
    bpb = np.broadcast_to(proj_b[None, :], (P, COUT))
    kwt = np.ascontiguousarray(k_W[nt].T.astype(bf))         # [COUT, N]
    kbrow = np.ascontiguousarray(k_b[nt][None, :].astype(bf))  # [1, N]
    VW = v_W[nt]                                             # [N, COUT]
    vb = v_b[nt]                                             # [N]

    in_maps = []
    for core in range(8):
        b, half = divmod(core, 2)
        jsl = slice(half * NJ, (half + 1) * NJ)
        xT = np.ascontiguousarray(node_rep[b].T.astype(bf))  # [CIN, N]
        # roll the i axis so this core's j-half occupies columns [0, NJ):
        # hn then indexes xt at fixed offsets; outp is un-rolled on host.
        xTr = np.ascontiguousarray(np.roll(xT, -half * NJ, axis=1))
        kwtr = np.ascontiguousarray(np.roll(kwt, -half * NJ, axis=1))
        kbrowr = np.ascontiguousarray(np.roll(kbrow, -half * NJ, axis=1))
        # [k, p, half, i]: per double-tile k, partition p holds its two
        # j rows (j = 256k + 128*half + p) contiguously -> one 2MB DMA.
        adjr = np.ascontiguousarray(
            np.roll(adjT[jsl, :], -half * NJ, axis=1)
            .reshape(NJT // 2, 2, P, N)
            .transpose(0, 2, 1, 3)
            .reshape(NJT // 2, P, 2 * N)
        )
        vw_h = VW[jsl]                                       # [NJ, COUT]
        vwn = np.ascontiguousarray(
            vw_h.reshape(NJT, P, COUT).transpose(1, 0, 2).reshape(P, NJT * COUT)
            .astype(bf)
        )
        vbcol = vb[jsl].reshape(NJT, P).T                    # [P, NJT]
        pkf = np.ascontiguousarray(
            np.concatenate([bpb, vbcol], axis=1).astype(f32)
        )
        in_maps.append({
            "adjt": adjr,
            "xt": xTr,
            "wpt": wpt,
            "bpcol": bpcol,
            "pkf": pkf,
            "kwt": kwtr,
            "kbrow": kbrowr,
            "vwn": vwn,
        })
    return in_maps


def kernel(node_rep, adj_matrix, node_type, proj_W, proj_b, k_W, k_b, v_W, v_b):
    global LAST_EXEC_NS, LAST_RESULTS
    in_maps = _prep_in_maps(
        node_rep, adj_matrix, node_type, proj_W, proj_b, k_W, k_b, v_W, v_b
    )
    nc = build_nc()
    trace = os.environ.get("KERNEL_TRACE", "0") == "1"
    res = run_bass_kernel_spmd(nc, in_maps, core_ids=list(range(8)), trace=trace)
    LAST_EXEC_NS = res.exec_time_ns
    LAST_RESULTS = res

    out = np.empty((B, N, COUT), dtype=np.float32)
    for b in range(B):
        m = None
        sg = None
        for half in range(2):
            r = res.results[2 * b + half]
            mp = np.asarray(r["outp"], dtype=np.float32)          # [COUT, N]
            mp = np.roll(mp, half * NJ, axis=1)  # un-roll the i axis
            sp = np.asarray(r["sgp"], dtype=np.float32).reshape(NJT, COUT).sum(axis=0)
            m = mp if m is None else m + mp
            sg = sp if sg is None else sg + sp
        out[b] = FIT_A * m.T + FIT_D * sg[None, :]
    return out
